# revision 19
# baseline (speedup 1.0000x reference)
"""Trainium2 Bass kernel for the attention-LSTM captioning RNN.

Problem (per full batch): x(64,128,512), A(64,1024,4,4), Wx(512,4096),
Wh(1024,4096), Wattn(1024,4096), b(4096) -> h-sequence (64,128,1024).

Strategy: data-parallel over N across 8 cores (8 samples/core, weights
replicated).  Per core:
  - precompute P[(n,l),g] = Af[n,:,l] @ Wattn  (PE, fp32) -> SBUF bf16
  - precompute xWx^T (gate-major) once (PE, f32r) -> SBUF bf16, indexed
    per step with a strided AP (no per-step DMA)
  - recurrence in transposed ("a^T") layout: gates live on 128 partitions
    (partition = gate-col % 128), batch (8) on the free dim.  Wh is the
    stationary operand (bf16, FWL), h^T the 8-wide moving operand.
    Attention is folded in as a second accumulating matmul with a
    block-diagonal softmax-weight matrix E (128x8) against stationary P.
Host numpy does all layout transposes (free: not timed on device).
"""

import math
import sys

sys.path.insert(0, "/root/shim")
sys.path.insert(0, "/opt/trn_rl_repo")

import numpy as np
import ml_dtypes

try:
    import antenv

    if "/root/shim/antenv" not in list(antenv.__path__):
        antenv.__path__.append("/root/shim/antenv")
except Exception:
    pass

import concourse.bass as bass
import concourse.bacc as bacc
import concourse.mybir as mybir
from concourse.tile import TileContext
from concourse.bass_utils import run_bass_kernel_spmd

FP32 = mybir.dt.float32
F32R = mybir.dt.float32r
BF16 = mybir.dt.bfloat16

# Problem constants (hardcoded per harness contract)
N, T, D, H = 64, 128, 512, 1024
NC = 8            # cores
NL = N // NC      # samples per core = 8
G = 4 * H         # 4096 gate columns
L = 16            # attention locations
HC = H // 128     # 8 h-chunks
GM = G // 128     # 32 gate-col chunks
DC = D // 128     # 4 d-chunks
INV_SQRT_H = 1.0 / math.sqrt(H)


def build_nc(timesteps=T):
    nc = bacc.Bacc()

    # ---- DRAM I/O (host-prepped layouts) ----
    xT_d = nc.dram_tensor("xT", [128, DC, NL, timesteps], FP32, kind="ExternalInput")
    afT_d = nc.dram_tensor("afT", [128, HC, NL, L], FP32, kind="ExternalInput")
    wx_d = nc.dram_tensor("wx", [128, DC, G], FP32, kind="ExternalInput")
    wh_d = nc.dram_tensor("wh", [128, HC, G], BF16, kind="ExternalInput")
    wattn_d = nc.dram_tensor("wattn", [128, HC, G], FP32, kind="ExternalInput")
    b_d = nc.dram_tensor("bias", [128, GM], FP32, kind="ExternalInput")
    mask_d = nc.dram_tensor("mask", [128, NL], FP32, kind="ExternalInput")
    out_d = nc.dram_tensor("hsT", [timesteps, 128, HC, NL], FP32, kind="ExternalOutput")

    with TileContext(nc) as tc:
        # ---------- persistent SBUF ----------
        with tc.tile_pool(name="persist", bufs=1) as pp:
            afTb = pp.tile([128, HC, NL, L], BF16)     # Af^T bf16, (hc,n,l) free
            p_sb = pp.tile([128, G], BF16)             # P[(n,l), g]
            wh_sb = pp.tile([128, HC, G], BF16)        # Wh tiles
            xwxt = pp.tile([128, GM, NL, timesteps], BF16)  # xWx^T (+bias)
            bias_sb = pp.tile([128, GM], FP32)
            mask_sb = pp.tile([128, NL], FP32)
            ones_sb = pp.tile([128, 1], FP32)          # for partition-sum matmul
            one1_sb = pp.tile([1, 1], FP32)            # identity for 1xF transpose
            onesr = pp.tile([1, NL * L], FP32)         # ones row (exp-via-tanh)
            hT32 = pp.tile([128, HC, NL], FP32)        # h^T fp32 (output + c path)
            hTb = pp.tile([128, HC, NL], BF16)         # h^T bf16 (matmul operand)
            cT = pp.tile([128, HC, NL], FP32)

            nc.sync.dma_start(bias_sb[:], b_d[:])
            nc.sync.dma_start(mask_sb[:], mask_d[:])
            nc.vector.memset(ones_sb[:], 1.0)
            nc.vector.memset(one1_sb[:], 1.0)
            nc.vector.memset(onesr[:], 1.0)

            # ---------- P = Af^T @ Wattn  (fp32, one-time) ----------
            with (
                tc.tile_pool(name="wattn", bufs=1) as wap,
                tc.tile_pool(name="wsl", bufs=2) as wslp,
                tc.tile_pool(name="ppsum", bufs=1, space="PSUM") as ppp,
            ):
                afT = wap.tile([128, HC, NL, L], FP32)
                nc.sync.dma_start(afT[:], afT_d[:])
                nc.vector.tensor_copy(afTb[:], afT[:])

                # h0 = mean over l of Af  (reduce innermost l)
                nc.vector.tensor_reduce(
                    hT32[:], afT[:], axis=mybir.AxisListType.X,
                    op=mybir.AluOpType.add,
                )
                nc.vector.tensor_scalar_mul(hT32[:], hT32[:], 2.0 / L)
                nc.vector.tensor_copy(cT[:], hT32[:])
                nc.vector.tensor_copy(hTb[:], hT32[:])

                pps = [ppp.tile([128, 1024], FP32, tag=f"pps{gc}", name=f"pps{gc}") for gc in range(4)]
                for hc in range(HC):
                    wsl = wslp.tile([128, G], FP32, tag="wsl")
                    nc.sync.dma_start(wsl[:], wattn_d[:, hc, :])
                    for gc in range(4):
                        for hf in range(2):
                            nc.tensor.matmul(
                                pps[gc][:, hf * 512:(hf + 1) * 512],
                                afT[:, hc, :, :],
                                wsl[
                                    :,
                                    gc * 1024 + hf * 512:gc * 1024 + (hf + 1) * 512,
                                ],
                                start=(hc == 0),
                                stop=(hc == HC - 1),
                            )
                for gc in range(4):
                    nc.vector.tensor_copy(
                        p_sb[:, gc * 1024:(gc + 1) * 1024], pps[gc][:]
                    )

            # ---------- xWx^T into SBUF bf16 (f32r, one-time) ----------
            with (
                tc.tile_pool(name="xwx", bufs=1) as xp,
                tc.tile_pool(name="xwxs", bufs=2) as xsp,
                tc.tile_pool(name="xwpsum", bufs=1, space="PSUM") as xpp,
            ):
                xT_r = xp.tile([128, DC, NL, timesteps], F32R)
                for dc in range(DC):
                    st2 = xsp.tile([128, NL * timesteps], FP32, tag="stage2")
                    nc.sync.dma_start(
                        st2[:], xT_d[:, dc, :, :].rearrange("p n t -> p (n t)")
                    )
                    nc.vector.tensor_copy(
                        xT_r[:, dc, :, :].rearrange("p n t -> p (n t)"), st2[:]
                    )
                ncols = NL * timesteps  # 1024
                col_chunks = [(s, min(s + 512, ncols)) for s in range(0, ncols, 512)]
                for mg in range(GM // 4):  # groups of 4 gate-chunks
                    xwg = [
                        xpp.tile([128, ncols], FP32, tag=f"xw{i}", name=f"xw{i}")
                        for i in range(4)
                    ]
                    for dc in range(DC):
                        st = xsp.tile([128, 512], FP32, tag="stage")
                        nc.sync.dma_start(
                            st[:], wx_d[:, dc, mg * 512:(mg + 1) * 512]
                        )
                        wxr = xsp.tile([128, 512], F32R, tag="wxr")
                        nc.vector.tensor_copy(wxr[:], st[:])
                        for i in range(4):
                            for (lo, hi) in col_chunks:
                                nc.tensor.matmul(
                                    xwg[i][:, lo:hi],
                                    wxr[:, i * 128:(i + 1) * 128],
                                    xT_r[:, dc, :, :].rearrange(
                                        "p n t -> p (n t)"
                                    )[:, lo:hi],
                                    start=(dc == 0),
                                    stop=(dc == DC - 1),
                                )
                    for i in range(4):
                        m = mg * 4 + i
                        nc.vector.tensor_scalar_add(
                            xwxt[:, m, :, :].rearrange("p n t -> p (n t)"),
                            xwg[i][:],
                            bias_sb[:, m:m + 1],
                        )

            # Wh load (bf16, direct)
            nc.sync.dma_start(wh_sb[:], wh_d[:])

            # ---------- recurrence ----------
            with (
                tc.tile_pool(name="step", bufs=2) as sp,
                tc.tile_pool(name="gpsum", bufs=2, space="PSUM") as gp,
                tc.tile_pool(name="spsum", bufs=2, space="PSUM") as ssp,
            ):
                with tc.For_i(0, timesteps, 1, staggered_reset=True) as ti:
                    aT = gp.tile([128, GM, NL], FP32, tag="aT")
                    uT = gp.tile([128, GM, NL], FP32, tag="uT")

                    def wh_block(ms):
                        for m in ms:
                            for kc in range(HC):
                                nc.tensor.matmul(
                                    aT[:, m, :],
                                    wh_sb[:, kc, m * 128:(m + 1) * 128],
                                    hTb[:, kc, :],
                                    start=(kc == 0),
                                    stop=(kc == HC - 1),
                                )

                    def e_block(ms):
                        for m in ms:
                            nc.tensor.matmul(
                                uT[:, m, :],
                                p_sb[:, m * 128:(m + 1) * 128],
                                ee[:],
                                start=True,
                                stop=True,
                            )

                    # ----- scores z-chain (DVE) emitted first: overlaps Wh MMs -----
                    z = sp.tile([128, NL, L, HC], FP32, tag="z")
                    nc.vector.tensor_tensor(
                        z[:],
                        afTb[:].transpose([0, 2, 3, 1]),   # (p, n, l, hc)
                        hTb[:].transpose([0, 2, 1]).unsqueeze(2).broadcast_to(
                            [128, NL, L, HC]
                        ),
                        mybir.AluOpType.mult,
                    )
                    z2 = sp.tile([128, NL * L], FP32, tag="z2")
                    nc.vector.tensor_reduce(
                        z2[:], z[:], axis=mybir.AxisListType.X, op=mybir.AluOpType.add
                    )

                    wh_block(range(0, 8))

                    sc = ssp.tile([1, NL * L], FP32, tag="sc")
                    nc.tensor.matmul(sc[:], ones_sb[:], z2[:], start=True, stop=True)

                    # softmax via tanh (no ACT table swap): e^x=(1+t)/(1-t), t=tanh(x/2)
                    th = sp.tile([1, NL * L], FP32, tag="th")
                    nc.scalar.activation(
                        th[:], sc[:], mybir.ActivationFunctionType.Tanh,
                        scale=INV_SQRT_H / 4.0,
                    )
                    den = sp.tile([1, NL * L], FP32, tag="den")
                    nc.vector.scalar_tensor_tensor(
                        den[:], th[:], -1.0, onesr[:],
                        mybir.AluOpType.mult, mybir.AluOpType.add,
                    )
                    rec = sp.tile([1, NL * L], FP32, tag="rec")
                    nc.vector.reciprocal(rec[:], den[:])
                    ex = sp.tile([1, NL * L], FP32, tag="ex")
                    nc.vector.scalar_tensor_tensor(
                        ex[:], th[:], 1.0, rec[:],
                        mybir.AluOpType.add, mybir.AluOpType.mult,
                    )
                    zs = sp.tile([1, NL], FP32, tag="zs")
                    nc.vector.tensor_reduce(
                        zs[:],
                        ex[:].rearrange("q (n l) -> q n l", n=NL),
                        axis=mybir.AxisListType.X,
                        op=mybir.AluOpType.add,
                    )
                    rz = sp.tile([1, NL], FP32, tag="rz")
                    nc.vector.reciprocal(rz[:], zs[:])
                    w = sp.tile([1, NL * L], FP32, tag="w")
                    nc.vector.tensor_tensor(
                        w[:].rearrange("q (n l) -> q n l", n=NL),
                        ex[:].rearrange("q (n l) -> q n l", n=NL),
                        rz[:].unsqueeze(2).broadcast_to([1, NL, L]),
                        mybir.AluOpType.mult,
                    )

                    wh_block(range(8, 16))

                    wT = ssp.tile([128, 1], FP32, tag="wT")
                    nc.tensor.transpose(wT[:], w[:], one1_sb[:])
                    ee = sp.tile([128, NL], BF16, tag="ee")
                    nc.vector.tensor_tensor(
                        ee[:],
                        mask_sb[:],
                        wT[:].broadcast_to([128, NL]),
                        mybir.AluOpType.mult,
                    )

                    wh_block(range(16, 24))

                    spre = sp.tile([128, GM, NL], FP32, tag="spre")
                    gs = sp.tile([128, GM, NL], FP32, tag="gs")
                    gl = gs[:].rearrange("p m n -> p (m n)")
                    q = HC * NL  # 64 columns per gate
                    gate_fn = [None] * 4

                    def gate_block(qi, scale):
                        lo, hi = qi * HC, (qi + 1) * HC
                        nc.vector.tensor_tensor(
                            spre[:, lo:hi, :].rearrange("p m n -> p (m n)").unsqueeze(2),
                            aT[:, lo:hi, :].rearrange("p m n -> p (m n)").unsqueeze(2),
                            xwxt[:, lo:hi, :, bass.ds(ti, 1)].rearrange(
                                "p m n t -> p (m n) t"
                            ),
                            mybir.AluOpType.add,
                        )
                        nc.vector.tensor_tensor(
                            spre[:, lo:hi, :], spre[:, lo:hi, :], uT[:, lo:hi, :],
                            mybir.AluOpType.add,
                        )
                        nc.scalar.activation(
                            gl[:, qi * q:(qi + 1) * q],
                            spre[:, lo:hi, :].rearrange("p m n -> p (m n)"),
                            mybir.ActivationFunctionType.Tanh, scale=scale,
                        )

                    e_block(range(0, 8))
                    gate_block(0, 1.0)          # i
                    wh_block(range(24, 32))
                    e_block(range(8, 16))
                    gate_block(1, 1.0)          # f
                    # c2-partial: t1 = (tf+1)*c2
                    t1 = sp.tile([128, HC * NL], FP32, tag="t1")
                    cflat = cT[:].rearrange("p c n -> p (c n)")
                    nc.vector.scalar_tensor_tensor(
                        t1[:], gl[:, 1 * q:2 * q], 1.0, cflat,
                        mybir.AluOpType.add, mybir.AluOpType.mult,
                    )
                    e_block(range(16, 24))
                    gate_block(2, 1.0)          # o
                    e_block(range(24, 32))
                    gate_block(3, 2.0)          # g
                    # c2' = 0.5*t1 + (ti+1)*tg
                    t2 = sp.tile([128, HC * NL], FP32, tag="t2")
                    nc.vector.scalar_tensor_tensor(
                        t2[:], gl[:, 0 * q:1 * q], 1.0, gl[:, 3 * q:4 * q],
                        mybir.AluOpType.add, mybir.AluOpType.mult,
                    )
                    nc.vector.scalar_tensor_tensor(
                        cflat, t1[:], 0.5, t2[:],
                        mybir.AluOpType.mult, mybir.AluOpType.add,
                    )
                    tc_t = sp.tile([128, HC * NL], FP32, tag="tct")
                    nc.scalar.activation(
                        tc_t[:], cflat, mybir.ActivationFunctionType.Tanh, scale=0.5
                    )
                    # h2 = (to+1)*tanh(c)
                    hflat = hT32[:].rearrange("p c n -> p (c n)")
                    nc.vector.scalar_tensor_tensor(
                        hflat, gl[:, 2 * q:3 * q], 1.0, tc_t[:],
                        mybir.AluOpType.add, mybir.AluOpType.mult,
                    )
                    nc.vector.tensor_copy(hTb[:], hT32[:])
                    nc.sync.dma_start(
                        out_d[bass.ds(ti, 1), :, :, :].rearrange(
                            "t p c n -> p (t c) n"
                        ),
                        hT32[:],
                    )

    nc.finalize()
    return nc


def prep_inputs(x, A, Wx, Wh, Wattn, b):
    """Host-side reshapes to device layouts; returns per-core input maps."""
    x = np.asarray(x, dtype=np.float32)
    A = np.asarray(A, dtype=np.float32)
    Wx = np.asarray(Wx, dtype=np.float32)
    Wh = np.asarray(Wh, dtype=np.float32)
    Wattn = np.asarray(Wattn, dtype=np.float32)
    b = np.asarray(b, dtype=np.float32)
    timesteps = x.shape[1]

    # weight layouts [p, kc, g] with k = kc*128 + p
    wx_h = np.ascontiguousarray(0.5 * Wx.reshape(DC, 128, G).transpose(1, 0, 2))
    wh_h = np.ascontiguousarray(
        (0.25 * Wh.reshape(HC, 128, G).transpose(1, 0, 2)).astype(ml_dtypes.bfloat16)
    )
    wattn_h = np.ascontiguousarray(0.5 * Wattn.reshape(HC, 128, G).transpose(1, 0, 2))
    b_h = np.ascontiguousarray(0.5 * b.reshape(GM, 128).T)  # [p, m]
    mask_h = np.zeros((128, NL), dtype=np.float32)
    for p in range(128):
        mask_h[p, p // L] = 1.0

    in_maps = []
    for c in range(NC):
        xs = x[c * NL:(c + 1) * NL]          # (8, T, 512)
        As = A[c * NL:(c + 1) * NL].reshape(NL, H, L)  # (8, 1024, 16)
        # xT [p, dc, n, t] = x[n, t, dc*128+p]
        xT_h = np.ascontiguousarray(
            xs.reshape(NL, timesteps, DC, 128).transpose(3, 2, 0, 1)
        )
        # afT [p, hc, n, l] = Af[n, hc*128+p, l]
        afT_h = np.ascontiguousarray(
            As.reshape(NL, HC, 128, L).transpose(2, 1, 0, 3)
        )
        in_maps.append(
            {
                "xT": xT_h,
                "afT": afT_h,
                "wx": wx_h,
                "wh": wh_h,
                "wattn": wattn_h,
                "bias": b_h,
                "mask": mask_h,
            }
        )
    return in_maps


_NC_CACHE = {}


def kernel(x, A, Wx, Wh, Wattn, b, trace=False):
    timesteps = x.shape[1]
    key = timesteps
    if key not in _NC_CACHE:
        _NC_CACHE[key] = build_nc(timesteps)
    nc = _NC_CACHE[key]
    in_maps = prep_inputs(x, A, Wx, Wh, Wattn, b)
    res = run_bass_kernel_spmd(nc, in_maps, list(range(NC)), trace=trace)
    outs = []
    for c in range(NC):
        hsT = res.results[c]["hsT"]  # (T, 128, HC, NL)
        # out[n, t, hc*128+p] = hsT[t, p, hc, n]
        outs.append(0.5 * hsT.transpose(3, 0, 2, 1).reshape(NL, timesteps, H))
    full = np.concatenate(outs, axis=0).astype(np.float32)
    kernel.last_result = res
    return full


# revision 20
# speedup vs baseline: 1.0980x; 1.0980x over previous
"""Trainium2 Bass kernel for the attention-LSTM captioning RNN.

Problem (per full batch): x(64,128,512), A(64,1024,4,4), Wx(512,4096),
Wh(1024,4096), Wattn(1024,4096), b(4096) -> h-sequence (64,128,1024).

Strategy: data-parallel over N across 8 cores (8 samples/core, weights
replicated).  Per core:
  - precompute P[(n,l),g] = Af[n,:,l] @ Wattn  (PE, fp32) -> SBUF bf16
  - precompute xWx^T (gate-major) once (PE, f32r) -> SBUF bf16, indexed
    per step with a strided AP (no per-step DMA)
  - recurrence in transposed ("a^T") layout: gates live on 128 partitions
    (partition = gate-col % 128), batch (8) on the free dim.  Wh is the
    stationary operand (bf16, FWL), h^T the 8-wide moving operand.
    Attention is folded in as a second accumulating matmul with a
    block-diagonal softmax-weight matrix E (128x8) against stationary P.
Host numpy does all layout transposes (free: not timed on device).
"""

import math
import sys

sys.path.insert(0, "/root/shim")
sys.path.insert(0, "/opt/trn_rl_repo")

import numpy as np
import ml_dtypes

try:
    import antenv

    if "/root/shim/antenv" not in list(antenv.__path__):
        antenv.__path__.append("/root/shim/antenv")
except Exception:
    pass

import concourse.bass as bass
import concourse.bacc as bacc
import concourse.mybir as mybir
from concourse.tile import TileContext
from concourse.bass_utils import run_bass_kernel_spmd

FP32 = mybir.dt.float32
F32R = mybir.dt.float32r
BF16 = mybir.dt.bfloat16

# Problem constants (hardcoded per harness contract)
N, T, D, H = 64, 128, 512, 1024
NC = 8            # cores
NL = N // NC      # samples per core = 8
G = 4 * H         # 4096 gate columns
L = 16            # attention locations
HC = H // 128     # 8 h-chunks
GM = G // 128     # 32 gate-col chunks
DC = D // 128     # 4 d-chunks
INV_SQRT_H = 1.0 / math.sqrt(H)


def build_nc(timesteps=T):
    nc = bacc.Bacc()

    # ---- DRAM I/O (host-prepped layouts) ----
    xT_d = nc.dram_tensor("xT", [128, DC, NL, timesteps], FP32, kind="ExternalInput")
    afT_d = nc.dram_tensor("afT", [128, HC, NL, L], FP32, kind="ExternalInput")
    wx_d = nc.dram_tensor("wx", [128, DC, G], FP32, kind="ExternalInput")
    wh_d = nc.dram_tensor("wh", [128, HC, G], BF16, kind="ExternalInput")
    wattn_d = nc.dram_tensor("wattn", [128, HC, G], FP32, kind="ExternalInput")
    b_d = nc.dram_tensor("bias", [128, GM], FP32, kind="ExternalInput")
    mask_d = nc.dram_tensor("mask", [128, NL], FP32, kind="ExternalInput")
    bmask_d = nc.dram_tensor("bmask", [128, 128], FP32, kind="ExternalInput")
    out_d = nc.dram_tensor("hsT", [timesteps, 128, HC, NL], BF16, kind="ExternalOutput")

    with TileContext(nc) as tc:
        # ---------- persistent SBUF ----------
        with tc.tile_pool(name="persist", bufs=1) as pp:
            afTb = pp.tile([128, HC, NL, L], BF16)     # Af^T bf16, (hc,n,l) free
            p_sb = pp.tile([128, G], BF16)             # P[(n,l), g]
            wh_sb = pp.tile([128, HC, G], BF16)        # Wh tiles
            xwxt = pp.tile([128, GM, NL, timesteps], BF16)  # xWx^T (+bias)
            bias_sb = pp.tile([128, GM], FP32)
            mask_sb = pp.tile([128, NL], FP32)
            ones_sb = pp.tile([128, 1], FP32)          # per-partition ones col
            bmask_sb = pp.tile([128, 128], FP32)       # 16-block partition mask
            hTb = pp.tile([128, HC, NL], BF16)         # h^T bf16 (2h)
            cT = pp.tile([128, HC, NL], FP32)

            nc.sync.dma_start(bias_sb[:], b_d[:])
            nc.sync.dma_start(mask_sb[:], mask_d[:])
            nc.vector.memset(ones_sb[:], 1.0)
            nc.sync.dma_start(bmask_sb[:], bmask_d[:])

            # ---------- P = Af^T @ Wattn  (fp32, one-time) ----------
            with (
                tc.tile_pool(name="wattn", bufs=1) as wap,
                tc.tile_pool(name="wsl", bufs=2) as wslp,
                tc.tile_pool(name="ppsum", bufs=1, space="PSUM") as ppp,
            ):
                afT = wap.tile([128, HC, NL, L], FP32)
                nc.sync.dma_start(afT[:], afT_d[:])
                nc.vector.tensor_copy(afTb[:], afT[:])

                # h0 = mean over l of Af  (h2 = 2*h convention)
                nc.vector.tensor_reduce(
                    cT[:], afT[:], axis=mybir.AxisListType.X,
                    op=mybir.AluOpType.add,
                )
                nc.vector.tensor_scalar_mul(cT[:], cT[:], 2.0 / L)
                nc.vector.tensor_copy(hTb[:], cT[:])

                pps = [ppp.tile([128, 1024], FP32, tag=f"pps{gc}", name=f"pps{gc}") for gc in range(4)]
                for hc in range(HC):
                    wsl = wslp.tile([128, G], FP32, tag="wsl")
                    nc.sync.dma_start(wsl[:], wattn_d[:, hc, :])
                    for gc in range(4):
                        for hf in range(2):
                            nc.tensor.matmul(
                                pps[gc][:, hf * 512:(hf + 1) * 512],
                                afT[:, hc, :, :],
                                wsl[
                                    :,
                                    gc * 1024 + hf * 512:gc * 1024 + (hf + 1) * 512,
                                ],
                                start=(hc == 0),
                                stop=(hc == HC - 1),
                            )
                for gc in range(4):
                    nc.vector.tensor_copy(
                        p_sb[:, gc * 1024:(gc + 1) * 1024], pps[gc][:]
                    )

            # ---------- xWx^T into SBUF bf16 (f32r, one-time) ----------
            with (
                tc.tile_pool(name="xwx", bufs=1) as xp,
                tc.tile_pool(name="xwxs", bufs=2) as xsp,
                tc.tile_pool(name="xwpsum", bufs=1, space="PSUM") as xpp,
            ):
                xT_r = xp.tile([128, DC, NL, timesteps], F32R)
                for dc in range(DC):
                    st2 = xsp.tile([128, NL * timesteps], FP32, tag="stage2")
                    nc.sync.dma_start(
                        st2[:], xT_d[:, dc, :, :].rearrange("p n t -> p (n t)")
                    )
                    nc.vector.tensor_copy(
                        xT_r[:, dc, :, :].rearrange("p n t -> p (n t)"), st2[:]
                    )
                ncols = NL * timesteps  # 1024
                col_chunks = [(s, min(s + 512, ncols)) for s in range(0, ncols, 512)]
                for mg in range(GM // 4):  # groups of 4 gate-chunks
                    xwg = [
                        xpp.tile([128, ncols], FP32, tag=f"xw{i}", name=f"xw{i}")
                        for i in range(4)
                    ]
                    for dc in range(DC):
                        st = xsp.tile([128, 512], FP32, tag="stage")
                        nc.sync.dma_start(
                            st[:], wx_d[:, dc, mg * 512:(mg + 1) * 512]
                        )
                        wxr = xsp.tile([128, 512], F32R, tag="wxr")
                        nc.vector.tensor_copy(wxr[:], st[:])
                        for i in range(4):
                            for (lo, hi) in col_chunks:
                                nc.tensor.matmul(
                                    xwg[i][:, lo:hi],
                                    wxr[:, i * 128:(i + 1) * 128],
                                    xT_r[:, dc, :, :].rearrange(
                                        "p n t -> p (n t)"
                                    )[:, lo:hi],
                                    start=(dc == 0),
                                    stop=(dc == DC - 1),
                                )
                    for i in range(4):
                        m = mg * 4 + i
                        nc.vector.tensor_scalar_add(
                            xwxt[:, m, :, :].rearrange("p n t -> p (n t)"),
                            xwg[i][:],
                            bias_sb[:, m:m + 1],
                        )

            # Wh load (bf16, direct)
            nc.sync.dma_start(wh_sb[:], wh_d[:])

            # ---------- recurrence ----------
            with (
                tc.tile_pool(name="step", bufs=2) as sp,
                tc.tile_pool(name="gpsum", bufs=2, space="PSUM") as gp,
                tc.tile_pool(name="spsum", bufs=2, space="PSUM") as ssp,
            ):
                with tc.For_i(0, timesteps, 1, staggered_reset=True) as ti:
                    aT = gp.tile([128, GM, NL], FP32, tag="aT")
                    uT = gp.tile([128, GM, NL], FP32, tag="uT")

                    def wh_block(ms):
                        for m in ms:
                            for kc in range(HC):
                                nc.tensor.matmul(
                                    aT[:, m, :],
                                    wh_sb[:, kc, m * 128:(m + 1) * 128],
                                    hTb[:, kc, :],
                                    start=(kc == 0),
                                    stop=(kc == HC - 1),
                                )

                    def e_block(ms):
                        for m in ms:
                            nc.tensor.matmul(
                                uT[:, m, :],
                                p_sb[:, m * 128:(m + 1) * 128],
                                ee[:],
                                start=True,
                                stop=True,
                            )

                    # ----- scores, partition-major: scp[(n,l), n'] -----
                    scp = ssp.tile([128, NL], FP32, tag="scp")
                    for kc in range(HC):
                        nc.tensor.matmul(
                            scp[:],
                            afTb[:, kc, :, :].rearrange("p n l -> p (n l)"),
                            hTb[:, kc, :],
                            start=(kc == 0),
                            stop=(kc == HC - 1),
                        )
                    # diagonal-block extract: s_col[p] = sum_n mask[p,n]*scp[p,n]
                    junk = sp.tile([128, NL], FP32, tag="junk")
                    s_col = sp.tile([128, 1], FP32, tag="s_col")
                    nc.vector.scalar_tensor_tensor(
                        junk[:], scp[:], 1.0, mask_sb[:],
                        mybir.AluOpType.mult, mybir.AluOpType.mult,
                        accum_out=s_col[:],
                    )
                    # exp via tanh: e = (1+t)/(1-t), t = tanh(s/(2*64*sqrtH^-1...))
                    th = sp.tile([128, 1], FP32, tag="th")
                    nc.scalar.activation(
                        th[:], s_col[:], mybir.ActivationFunctionType.Tanh,
                        scale=INV_SQRT_H / 4.0,
                    )
                    den = sp.tile([128, 1], FP32, tag="den")
                    nc.vector.scalar_tensor_tensor(
                        den[:], th[:], -1.0, ones_sb[:],
                        mybir.AluOpType.mult, mybir.AluOpType.add,
                    )
                    rec = sp.tile([128, 1], FP32, tag="rec")
                    nc.vector.reciprocal(rec[:], den[:])
                    e_col = sp.tile([128, 1], FP32, tag="e_col")
                    nc.vector.scalar_tensor_tensor(
                        e_col[:], th[:], 1.0, rec[:],
                        mybir.AluOpType.add, mybir.AluOpType.mult,
                    )

                    wh_block(range(0, 8))

                    # per-sample sums replicated partition-major
                    zsp = ssp.tile([128, 1], FP32, tag="zsp")
                    nc.tensor.matmul(
                        zsp[:], bmask_sb[:], e_col[:], start=True, stop=True
                    )
                    rz = sp.tile([128, 1], FP32, tag="rz")
                    nc.vector.reciprocal(rz[:], zsp[:])
                    en = sp.tile([128, 1], FP32, tag="en")
                    nc.vector.tensor_tensor(
                        en[:], e_col[:], rz[:], mybir.AluOpType.mult
                    )
                    ee = sp.tile([128, NL], BF16, tag="ee")
                    nc.vector.tensor_scalar_mul(ee[:], mask_sb[:], en[:])

                    wh_block(range(8, 16))

                    spre = sp.tile([128, GM, NL], FP32, tag="spre")
                    gs = sp.tile([128, GM, NL], FP32, tag="gs")
                    gl = gs[:].rearrange("p m n -> p (m n)")
                    q = HC * NL  # 64 columns per gate

                    def gate_block(qi, scale):
                        lo, hi = qi * HC, (qi + 1) * HC
                        nc.vector.tensor_tensor(
                            spre[:, lo:hi, :].rearrange("p m n -> p (m n)").unsqueeze(2),
                            aT[:, lo:hi, :].rearrange("p m n -> p (m n)").unsqueeze(2),
                            xwxt[:, lo:hi, :, bass.ds(ti, 1)].rearrange(
                                "p m n t -> p (m n) t"
                            ),
                            mybir.AluOpType.add,
                        )
                        nc.vector.tensor_tensor(
                            spre[:, lo:hi, :], spre[:, lo:hi, :], uT[:, lo:hi, :],
                            mybir.AluOpType.add,
                        )
                        nc.scalar.activation(
                            gl[:, qi * q:(qi + 1) * q],
                            spre[:, lo:hi, :].rearrange("p m n -> p (m n)"),
                            mybir.ActivationFunctionType.Tanh, scale=scale,
                        )

                    e_block(range(0, 8))
                    gate_block(0, 1.0)          # i
                    wh_block(range(16, 24))
                    e_block(range(8, 16))
                    gate_block(1, 1.0)          # f
                    t1 = sp.tile([128, HC * NL], FP32, tag="t1")
                    cflat = cT[:].rearrange("p c n -> p (c n)")
                    nc.vector.scalar_tensor_tensor(
                        t1[:], gl[:, 1 * q:2 * q], 1.0, cflat,
                        mybir.AluOpType.add, mybir.AluOpType.mult,
                    )
                    wh_block(range(24, 32))
                    e_block(range(16, 24))
                    gate_block(2, 1.0)          # o
                    e_block(range(24, 32))
                    gate_block(3, 2.0)          # g
                    # c2' = 0.5*t1 + (ti+1)*tg
                    t2 = sp.tile([128, HC * NL], FP32, tag="t2")
                    nc.vector.scalar_tensor_tensor(
                        t2[:], gl[:, 0 * q:1 * q], 1.0, gl[:, 3 * q:4 * q],
                        mybir.AluOpType.add, mybir.AluOpType.mult,
                    )
                    nc.vector.scalar_tensor_tensor(
                        cflat, t1[:], 0.5, t2[:],
                        mybir.AluOpType.mult, mybir.AluOpType.add,
                    )
                    tc_t = sp.tile([128, HC * NL], FP32, tag="tct")
                    nc.scalar.activation(
                        tc_t[:], cflat, mybir.ActivationFunctionType.Tanh, scale=0.5
                    )
                    # h2 = (to+1)*tanh(c)  (bf16 directly)
                    nc.vector.scalar_tensor_tensor(
                        hTb[:].rearrange("p c n -> p (c n)"),
                        gl[:, 2 * q:3 * q], 1.0, tc_t[:],
                        mybir.AluOpType.add, mybir.AluOpType.mult,
                    )
                    nc.sync.dma_start(
                        out_d[bass.ds(ti, 1), :, :, :].rearrange(
                            "t p c n -> p (t c) n"
                        ),
                        hTb[:],
                    )

    nc.finalize()
    return nc


def prep_inputs(x, A, Wx, Wh, Wattn, b):
    """Host-side reshapes to device layouts; returns per-core input maps."""
    x = np.asarray(x, dtype=np.float32)
    A = np.asarray(A, dtype=np.float32)
    Wx = np.asarray(Wx, dtype=np.float32)
    Wh = np.asarray(Wh, dtype=np.float32)
    Wattn = np.asarray(Wattn, dtype=np.float32)
    b = np.asarray(b, dtype=np.float32)
    timesteps = x.shape[1]

    # weight layouts [p, kc, g] with k = kc*128 + p
    wx_h = np.ascontiguousarray(0.5 * Wx.reshape(DC, 128, G).transpose(1, 0, 2))
    wh_h = np.ascontiguousarray(
        (0.25 * Wh.reshape(HC, 128, G).transpose(1, 0, 2)).astype(ml_dtypes.bfloat16)
    )
    wattn_h = np.ascontiguousarray(0.5 * Wattn.reshape(HC, 128, G).transpose(1, 0, 2))
    b_h = np.ascontiguousarray(0.5 * b.reshape(GM, 128).T)  # [p, m]
    mask_h = np.zeros((128, NL), dtype=np.float32)
    for p in range(128):
        mask_h[p, p // L] = 1.0
    bmask_h = (np.arange(128)[:, None] // L == np.arange(128)[None, :] // L).astype(
        np.float32
    )

    in_maps = []
    for c in range(NC):
        xs = x[c * NL:(c + 1) * NL]          # (8, T, 512)
        As = A[c * NL:(c + 1) * NL].reshape(NL, H, L)  # (8, 1024, 16)
        # xT [p, dc, n, t] = x[n, t, dc*128+p]
        xT_h = np.ascontiguousarray(
            xs.reshape(NL, timesteps, DC, 128).transpose(3, 2, 0, 1)
        )
        # afT [p, hc, n, l] = Af[n, hc*128+p, l]
        afT_h = np.ascontiguousarray(
            As.reshape(NL, HC, 128, L).transpose(2, 1, 0, 3)
        )
        in_maps.append(
            {
                "xT": xT_h,
                "afT": afT_h,
                "wx": wx_h,
                "wh": wh_h,
                "wattn": wattn_h,
                "bias": b_h,
                "mask": mask_h,
                "bmask": bmask_h,
            }
        )
    return in_maps


_NC_CACHE = {}


def kernel(x, A, Wx, Wh, Wattn, b, trace=False):
    timesteps = x.shape[1]
    key = timesteps
    if key not in _NC_CACHE:
        _NC_CACHE[key] = build_nc(timesteps)
    nc = _NC_CACHE[key]
    in_maps = prep_inputs(x, A, Wx, Wh, Wattn, b)
    res = run_bass_kernel_spmd(nc, in_maps, list(range(NC)), trace=trace)
    outs = []
    for c in range(NC):
        hsT = res.results[c]["hsT"]  # (T, 128, HC, NL)
        # out[n, t, hc*128+p] = hsT[t, p, hc, n]
        outs.append(0.5 * hsT.astype(np.float32).transpose(3, 0, 2, 1).reshape(NL, timesteps, H))
    full = np.concatenate(outs, axis=0).astype(np.float32)
    kernel.last_result = res
    return full


# revision 21
# speedup vs baseline: 1.1357x; 1.0343x over previous
"""Trainium2 Bass kernel for the attention-LSTM captioning RNN.

Problem (per full batch): x(64,128,512), A(64,1024,4,4), Wx(512,4096),
Wh(1024,4096), Wattn(1024,4096), b(4096) -> h-sequence (64,128,1024).

Strategy: data-parallel over N across 8 cores (8 samples/core, weights
replicated).  Per core:
  - precompute P[(n,l),g] = Af[n,:,l] @ Wattn  (PE, fp32) -> SBUF bf16
  - precompute xWx^T (gate-major) once (PE, f32r) -> SBUF bf16, indexed
    per step with a strided AP (no per-step DMA)
  - recurrence in transposed ("a^T") layout: gates live on 128 partitions
    (partition = gate-col % 128), batch (8) on the free dim.  Wh is the
    stationary operand (bf16, FWL), h^T the 8-wide moving operand.
    Attention is folded in as a second accumulating matmul with a
    block-diagonal softmax-weight matrix E (128x8) against stationary P.
Host numpy does all layout transposes (free: not timed on device).
"""

import math
import sys

sys.path.insert(0, "/root/shim")
sys.path.insert(0, "/opt/trn_rl_repo")

import numpy as np
import ml_dtypes

try:
    import antenv

    if "/root/shim/antenv" not in list(antenv.__path__):
        antenv.__path__.append("/root/shim/antenv")
except Exception:
    pass

import concourse.bass as bass
import concourse.bacc as bacc
import concourse.mybir as mybir
from concourse.tile import TileContext
from concourse.bass_utils import run_bass_kernel_spmd

FP32 = mybir.dt.float32
F32R = mybir.dt.float32r
BF16 = mybir.dt.bfloat16

# Problem constants (hardcoded per harness contract)
N, T, D, H = 64, 128, 512, 1024
NC = 8            # cores
NL = N // NC      # samples per core = 8
G = 4 * H         # 4096 gate columns
L = 16            # attention locations
HC = H // 128     # 8 h-chunks
GM = G // 128     # 32 gate-col chunks
DC = D // 128     # 4 d-chunks
INV_SQRT_H = 1.0 / math.sqrt(H)


def build_nc(timesteps=T):
    nc = bacc.Bacc()

    # ---- DRAM I/O (host-prepped layouts) ----
    xT_d = nc.dram_tensor("xT", [128, DC, NL, timesteps], FP32, kind="ExternalInput")
    afT_d = nc.dram_tensor("afT", [128, HC, NL, L], FP32, kind="ExternalInput")
    wx_d = nc.dram_tensor("wx", [128, DC, G], FP32, kind="ExternalInput")
    wh_d = nc.dram_tensor("wh", [128, HC, G], BF16, kind="ExternalInput")
    wattn_d = nc.dram_tensor("wattn", [128, HC, G], FP32, kind="ExternalInput")
    b_d = nc.dram_tensor("bias", [128, GM], FP32, kind="ExternalInput")
    mask_d = nc.dram_tensor("mask", [128, NL], FP32, kind="ExternalInput")
    bmask_d = nc.dram_tensor("bmask", [128, 128], FP32, kind="ExternalInput")
    out_d = nc.dram_tensor("hsT", [timesteps, 128, HC, NL], BF16, kind="ExternalOutput")

    with TileContext(nc) as tc:
        # ---------- persistent SBUF ----------
        with tc.tile_pool(name="persist", bufs=1) as pp:
            afTb = pp.tile([128, HC, NL, L], BF16)     # Af^T bf16, (hc,n,l) free
            p_sb = pp.tile([128, G], BF16)             # P[(n,l), g]
            wh_sb = pp.tile([128, HC, G], BF16)        # Wh tiles
            xwxt = pp.tile([128, GM, NL, timesteps], BF16)  # xWx^T (+bias)
            bias_sb = pp.tile([128, GM], FP32)
            mask_sb = pp.tile([128, NL], FP32)
            ones_sb = pp.tile([128, 1], FP32)          # per-partition ones col
            bmask_sb = pp.tile([128, 128], FP32)       # 16-block partition mask
            hTb = pp.tile([128, HC, NL], BF16)         # h^T bf16 (2h)
            cT = pp.tile([128, HC, NL], FP32)

            nc.sync.dma_start(bias_sb[:], b_d[:])
            nc.sync.dma_start(mask_sb[:], mask_d[:])
            nc.vector.memset(ones_sb[:], 1.0)
            nc.sync.dma_start(bmask_sb[:], bmask_d[:])

            # ---------- P = Af^T @ Wattn  (fp32, one-time) ----------
            with (
                tc.tile_pool(name="wattn", bufs=1) as wap,
                tc.tile_pool(name="wsl", bufs=2) as wslp,
                tc.tile_pool(name="ppsum", bufs=1, space="PSUM") as ppp,
            ):
                afT = wap.tile([128, HC, NL, L], FP32)
                nc.sync.dma_start(afT[:], afT_d[:])
                nc.vector.tensor_copy(afTb[:], afT[:])

                # h0 = mean over l of Af  (h2 = 2*h convention)
                nc.vector.tensor_reduce(
                    cT[:], afT[:], axis=mybir.AxisListType.X,
                    op=mybir.AluOpType.add,
                )
                nc.vector.tensor_scalar_mul(cT[:], cT[:], 2.0 / L)
                nc.vector.tensor_copy(hTb[:], cT[:])

                pps = [ppp.tile([128, 1024], FP32, tag=f"pps{gc}", name=f"pps{gc}") for gc in range(4)]
                for hc in range(HC):
                    wsl = wslp.tile([128, G], FP32, tag="wsl")
                    nc.sync.dma_start(wsl[:], wattn_d[:, hc, :])
                    for gc in range(4):
                        for hf in range(2):
                            nc.tensor.matmul(
                                pps[gc][:, hf * 512:(hf + 1) * 512],
                                afT[:, hc, :, :],
                                wsl[
                                    :,
                                    gc * 1024 + hf * 512:gc * 1024 + (hf + 1) * 512,
                                ],
                                start=(hc == 0),
                                stop=(hc == HC - 1),
                            )
                for gc in range(4):
                    nc.vector.tensor_copy(
                        p_sb[:, gc * 1024:(gc + 1) * 1024], pps[gc][:]
                    )

            # ---------- xWx^T into SBUF bf16 (f32r, one-time) ----------
            with (
                tc.tile_pool(name="xwx", bufs=1) as xp,
                tc.tile_pool(name="xwxs", bufs=2) as xsp,
                tc.tile_pool(name="xwpsum", bufs=1, space="PSUM") as xpp,
            ):
                xT_r = xp.tile([128, DC, NL, timesteps], F32R)
                for dc in range(DC):
                    st2 = xsp.tile([128, NL * timesteps], FP32, tag="stage2")
                    nc.sync.dma_start(
                        st2[:], xT_d[:, dc, :, :].rearrange("p n t -> p (n t)")
                    )
                    nc.vector.tensor_copy(
                        xT_r[:, dc, :, :].rearrange("p n t -> p (n t)"), st2[:]
                    )
                ncols = NL * timesteps  # 1024
                col_chunks = [(s, min(s + 512, ncols)) for s in range(0, ncols, 512)]
                for mg in range(GM // 4):  # groups of 4 gate-chunks
                    xwg = [
                        xpp.tile([128, ncols], FP32, tag=f"xw{i}", name=f"xw{i}")
                        for i in range(4)
                    ]
                    for dc in range(DC):
                        st = xsp.tile([128, 512], FP32, tag="stage")
                        nc.sync.dma_start(
                            st[:], wx_d[:, dc, mg * 512:(mg + 1) * 512]
                        )
                        wxr = xsp.tile([128, 512], F32R, tag="wxr")
                        nc.vector.tensor_copy(wxr[:], st[:])
                        for i in range(4):
                            for (lo, hi) in col_chunks:
                                nc.tensor.matmul(
                                    xwg[i][:, lo:hi],
                                    wxr[:, i * 128:(i + 1) * 128],
                                    xT_r[:, dc, :, :].rearrange(
                                        "p n t -> p (n t)"
                                    )[:, lo:hi],
                                    start=(dc == 0),
                                    stop=(dc == DC - 1),
                                )
                    for i in range(4):
                        m = mg * 4 + i
                        nc.vector.tensor_scalar_add(
                            xwxt[:, m, :, :].rearrange("p n t -> p (n t)"),
                            xwg[i][:],
                            bias_sb[:, m:m + 1],
                        )

            # Wh load (bf16, direct)
            nc.sync.dma_start(wh_sb[:], wh_d[:])

            # ---------- recurrence ----------
            with (
                tc.tile_pool(name="step", bufs=2) as sp,
                tc.tile_pool(name="gpsum", bufs=1, space="PSUM") as gp,
                tc.tile_pool(name="spsum", bufs=1, space="PSUM") as ssp,
            ):
                with tc.For_i(0, timesteps, 1, staggered_reset=True) as ti:
                    aT = gp.tile([128, GM, NL], FP32, tag="aT")
                    uTq = [
                        gp.tile([128, HC, NL], FP32, tag=f"uT{i}", name=f"uT{i}")
                        for i in range(4)
                    ]

                    def wh_block(ms):
                        for m in ms:
                            for kc in range(HC):
                                nc.tensor.matmul(
                                    aT[:, m, :],
                                    wh_sb[:, kc, m * 128:(m + 1) * 128],
                                    hTb[:, kc, :],
                                    start=(kc == 0),
                                    stop=(kc == HC - 1),
                                )

                    def e_block(qi):
                        for j in range(HC):
                            m = qi * HC + j
                            nc.tensor.matmul(
                                uTq[qi][:, j, :],
                                p_sb[:, m * 128:(m + 1) * 128],
                                ee[:],
                                start=True,
                                stop=True,
                            )

                    # ----- scores, partition-major: scp[(n,l), n'] -----
                    scp = ssp.tile([128, NL], FP32, tag="scp")
                    for kc in range(HC):
                        nc.tensor.matmul(
                            scp[:],
                            afTb[:, kc, :, :].rearrange("p n l -> p (n l)"),
                            hTb[:, kc, :],
                            start=(kc == 0),
                            stop=(kc == HC - 1),
                        )
                    # diagonal-block extract: s_col[p] = sum_n mask[p,n]*scp[p,n]
                    junk = sp.tile([128, NL], FP32, tag="junk")
                    s_col = sp.tile([128, 1], FP32, tag="s_col")
                    nc.vector.scalar_tensor_tensor(
                        junk[:], scp[:], 1.0, mask_sb[:],
                        mybir.AluOpType.mult, mybir.AluOpType.mult,
                        accum_out=s_col[:],
                    )
                    # exp via tanh: e = (1+t)/(1-t), t = tanh(s/(2*64*sqrtH^-1...))
                    th = sp.tile([128, 1], FP32, tag="th")
                    nc.scalar.activation(
                        th[:], s_col[:], mybir.ActivationFunctionType.Tanh,
                        scale=INV_SQRT_H / 4.0,
                    )
                    den = sp.tile([128, 1], FP32, tag="den")
                    nc.vector.scalar_tensor_tensor(
                        den[:], th[:], -1.0, ones_sb[:],
                        mybir.AluOpType.mult, mybir.AluOpType.add,
                    )
                    rec = sp.tile([128, 1], FP32, tag="rec")
                    nc.vector.reciprocal(rec[:], den[:])
                    e_col = sp.tile([128, 1], FP32, tag="e_col")
                    nc.vector.scalar_tensor_tensor(
                        e_col[:], th[:], 1.0, rec[:],
                        mybir.AluOpType.add, mybir.AluOpType.mult,
                    )

                    wh_block(range(0, 8))

                    # per-sample sums replicated partition-major
                    zsp = ssp.tile([128, 1], FP32, tag="zsp")
                    nc.tensor.matmul(
                        zsp[:], bmask_sb[:], e_col[:], start=True, stop=True
                    )
                    rz = sp.tile([128, 1], FP32, tag="rz")
                    nc.vector.reciprocal(rz[:], zsp[:])
                    en = sp.tile([128, 1], FP32, tag="en")
                    nc.vector.tensor_tensor(
                        en[:], e_col[:], rz[:], mybir.AluOpType.mult
                    )
                    ee = sp.tile([128, NL], BF16, tag="ee")
                    nc.vector.tensor_scalar_mul(ee[:], mask_sb[:], en[:])

                    wh_block(range(8, 32))

                    spre = sp.tile([128, GM, NL], FP32, tag="spre")
                    gs = sp.tile([128, GM, NL], FP32, tag="gs")
                    gl = gs[:].rearrange("p m n -> p (m n)")
                    q = HC * NL  # 64 columns per gate

                    def gate_block(qi, scale):
                        lo, hi = qi * HC, (qi + 1) * HC
                        nc.vector.tensor_tensor(
                            spre[:, lo:hi, :].rearrange("p m n -> p (m n)").unsqueeze(2),
                            aT[:, lo:hi, :].rearrange("p m n -> p (m n)").unsqueeze(2),
                            xwxt[:, lo:hi, :, bass.ds(ti, 1)].rearrange(
                                "p m n t -> p (m n) t"
                            ),
                            mybir.AluOpType.add,
                        )
                        nc.vector.tensor_tensor(
                            spre[:, lo:hi, :], spre[:, lo:hi, :], uTq[qi][:],
                            mybir.AluOpType.add,
                        )
                        nc.scalar.activation(
                            gl[:, qi * q:(qi + 1) * q],
                            spre[:, lo:hi, :].rearrange("p m n -> p (m n)"),
                            mybir.ActivationFunctionType.Tanh, scale=scale,
                        )

                    e_block(0)
                    gate_block(0, 1.0)          # i
                    e_block(1)
                    gate_block(1, 1.0)          # f
                    t1 = sp.tile([128, HC * NL], FP32, tag="t1")
                    cflat = cT[:].rearrange("p c n -> p (c n)")
                    nc.vector.scalar_tensor_tensor(
                        t1[:], gl[:, 1 * q:2 * q], 1.0, cflat,
                        mybir.AluOpType.add, mybir.AluOpType.mult,
                    )
                    e_block(2)
                    gate_block(2, 1.0)          # o
                    e_block(3)
                    gate_block(3, 2.0)          # g
                    # c2' = 0.5*t1 + (ti+1)*tg
                    t2 = sp.tile([128, HC * NL], FP32, tag="t2")
                    nc.vector.scalar_tensor_tensor(
                        t2[:], gl[:, 0 * q:1 * q], 1.0, gl[:, 3 * q:4 * q],
                        mybir.AluOpType.add, mybir.AluOpType.mult,
                    )
                    nc.vector.scalar_tensor_tensor(
                        cflat, t1[:], 0.5, t2[:],
                        mybir.AluOpType.mult, mybir.AluOpType.add,
                    )
                    tc_t = sp.tile([128, HC * NL], FP32, tag="tct")
                    nc.scalar.activation(
                        tc_t[:], cflat, mybir.ActivationFunctionType.Tanh, scale=0.5
                    )
                    # h2 = (to+1)*tanh(c)  (bf16 directly)
                    nc.vector.scalar_tensor_tensor(
                        hTb[:].rearrange("p c n -> p (c n)"),
                        gl[:, 2 * q:3 * q], 1.0, tc_t[:],
                        mybir.AluOpType.add, mybir.AluOpType.mult,
                    )
                    nc.sync.dma_start(
                        out_d[bass.ds(ti, 1), :, :, :].rearrange(
                            "t p c n -> p (t c) n"
                        ),
                        hTb[:],
                    )

    nc.finalize()
    return nc


def prep_inputs(x, A, Wx, Wh, Wattn, b):
    """Host-side reshapes to device layouts; returns per-core input maps."""
    x = np.asarray(x, dtype=np.float32)
    A = np.asarray(A, dtype=np.float32)
    Wx = np.asarray(Wx, dtype=np.float32)
    Wh = np.asarray(Wh, dtype=np.float32)
    Wattn = np.asarray(Wattn, dtype=np.float32)
    b = np.asarray(b, dtype=np.float32)
    timesteps = x.shape[1]

    # weight layouts [p, kc, g] with k = kc*128 + p
    wx_h = np.ascontiguousarray(0.5 * Wx.reshape(DC, 128, G).transpose(1, 0, 2))
    wh_h = np.ascontiguousarray(
        (0.25 * Wh.reshape(HC, 128, G).transpose(1, 0, 2)).astype(ml_dtypes.bfloat16)
    )
    wattn_h = np.ascontiguousarray(0.5 * Wattn.reshape(HC, 128, G).transpose(1, 0, 2))
    b_h = np.ascontiguousarray(0.5 * b.reshape(GM, 128).T)  # [p, m]
    mask_h = np.zeros((128, NL), dtype=np.float32)
    for p in range(128):
        mask_h[p, p // L] = 1.0
    bmask_h = (np.arange(128)[:, None] // L == np.arange(128)[None, :] // L).astype(
        np.float32
    )

    in_maps = []
    for c in range(NC):
        xs = x[c * NL:(c + 1) * NL]          # (8, T, 512)
        As = A[c * NL:(c + 1) * NL].reshape(NL, H, L)  # (8, 1024, 16)
        # xT [p, dc, n, t] = x[n, t, dc*128+p]
        xT_h = np.ascontiguousarray(
            xs.reshape(NL, timesteps, DC, 128).transpose(3, 2, 0, 1)
        )
        # afT [p, hc, n, l] = Af[n, hc*128+p, l]
        afT_h = np.ascontiguousarray(
            As.reshape(NL, HC, 128, L).transpose(2, 1, 0, 3)
        )
        in_maps.append(
            {
                "xT": xT_h,
                "afT": afT_h,
                "wx": wx_h,
                "wh": wh_h,
                "wattn": wattn_h,
                "bias": b_h,
                "mask": mask_h,
                "bmask": bmask_h,
            }
        )
    return in_maps


_NC_CACHE = {}


def kernel(x, A, Wx, Wh, Wattn, b, trace=False):
    timesteps = x.shape[1]
    key = timesteps
    if key not in _NC_CACHE:
        _NC_CACHE[key] = build_nc(timesteps)
    nc = _NC_CACHE[key]
    in_maps = prep_inputs(x, A, Wx, Wh, Wattn, b)
    res = run_bass_kernel_spmd(nc, in_maps, list(range(NC)), trace=trace)
    outs = []
    for c in range(NC):
        hsT = res.results[c]["hsT"]  # (T, 128, HC, NL)
        # out[n, t, hc*128+p] = hsT[t, p, hc, n]
        outs.append(0.5 * hsT.astype(np.float32).transpose(3, 0, 2, 1).reshape(NL, timesteps, H))
    full = np.concatenate(outs, axis=0).astype(np.float32)
    kernel.last_result = res
    return full


# revision 22
# speedup vs baseline: 1.1522x; 1.0146x over previous
"""Trainium2 Bass kernel for the attention-LSTM captioning RNN.

Problem (per full batch): x(64,128,512), A(64,1024,4,4), Wx(512,4096),
Wh(1024,4096), Wattn(1024,4096), b(4096) -> h-sequence (64,128,1024).

Strategy: data-parallel over N across 8 cores (8 samples/core, weights
replicated).  Per core:
  - precompute P[(n,l),g] = Af[n,:,l] @ Wattn  (PE, fp32) -> SBUF bf16
  - precompute xWx^T (gate-major) once (PE, f32r) -> SBUF bf16, indexed
    per step with a strided AP (no per-step DMA)
  - recurrence in transposed ("a^T") layout: gates live on 128 partitions
    (partition = gate-col % 128), batch (8) on the free dim.  Wh is the
    stationary operand (bf16, FWL), h^T the 8-wide moving operand.
    Attention is folded in as a second accumulating matmul with a
    block-diagonal softmax-weight matrix E (128x8) against stationary P.
Host numpy does all layout transposes (free: not timed on device).
"""

import math
import sys

sys.path.insert(0, "/root/shim")
sys.path.insert(0, "/opt/trn_rl_repo")

import numpy as np
import ml_dtypes

try:
    import antenv

    if "/root/shim/antenv" not in list(antenv.__path__):
        antenv.__path__.append("/root/shim/antenv")
except Exception:
    pass

import concourse.bass as bass
import concourse.bacc as bacc
import concourse.mybir as mybir
from concourse.tile import TileContext
from concourse.bass_utils import run_bass_kernel_spmd

FP32 = mybir.dt.float32
F32R = mybir.dt.float32r
BF16 = mybir.dt.bfloat16

# Problem constants (hardcoded per harness contract)
N, T, D, H = 64, 128, 512, 1024
NC = 8            # cores
NL = N // NC      # samples per core = 8
G = 4 * H         # 4096 gate columns
L = 16            # attention locations
HC = H // 128     # 8 h-chunks
GM = G // 128     # 32 gate-col chunks
DC = D // 128     # 4 d-chunks
INV_SQRT_H = 1.0 / math.sqrt(H)


def build_nc(timesteps=T):
    nc = bacc.Bacc()

    # ---- DRAM I/O (host-prepped layouts) ----
    xT_d = nc.dram_tensor("xT", [128, DC, NL, timesteps], FP32, kind="ExternalInput")
    afT_d = nc.dram_tensor("afT", [128, HC, NL, L], FP32, kind="ExternalInput")
    wx_d = nc.dram_tensor("wx", [128, DC, G], FP32, kind="ExternalInput")
    wh_d = nc.dram_tensor("wh", [128, HC, G], BF16, kind="ExternalInput")
    wattn_d = nc.dram_tensor("wattn", [128, HC, G], FP32, kind="ExternalInput")
    b_d = nc.dram_tensor("bias", [128, GM], FP32, kind="ExternalInput")
    mask_d = nc.dram_tensor("mask", [128, NL], FP32, kind="ExternalInput")
    bmask_d = nc.dram_tensor("bmask", [128, 128], FP32, kind="ExternalInput")
    out_d = nc.dram_tensor("hsT", [timesteps, 128, HC, NL], BF16, kind="ExternalOutput")

    with TileContext(nc) as tc:
        # ---------- persistent SBUF ----------
        with tc.tile_pool(name="persist", bufs=1) as pp:
            afTb = pp.tile([128, HC, NL, L], BF16)     # Af^T bf16, (hc,n,l) free
            p_sb = pp.tile([128, G], BF16)             # P[(n,l), g]
            wh_sb = pp.tile([128, HC, G], BF16)        # Wh tiles
            xwxt = pp.tile([128, GM, NL, timesteps], BF16)  # xWx^T (+bias)
            bias_sb = pp.tile([128, GM], FP32)
            mask_sb = pp.tile([128, NL], FP32)
            ones_sb = pp.tile([128, 1], FP32)          # per-partition ones col
            bmask_sb = pp.tile([128, 128], FP32)       # 16-block partition mask
            hTb = pp.tile([128, HC, NL], BF16)         # h^T bf16 (2h)
            cT = pp.tile([128, HC, NL], FP32)

            nc.sync.dma_start(bias_sb[:], b_d[:])
            nc.sync.dma_start(mask_sb[:], mask_d[:])
            nc.vector.memset(ones_sb[:], 1.0)
            nc.sync.dma_start(bmask_sb[:], bmask_d[:])

            # ---------- P = Af^T @ Wattn  (fp32, one-time) ----------
            with (
                tc.tile_pool(name="wattn", bufs=1) as wap,
                tc.tile_pool(name="wsl", bufs=2) as wslp,
                tc.tile_pool(name="ppsum", bufs=1, space="PSUM") as ppp,
            ):
                afT = wap.tile([128, HC, NL, L], FP32)
                nc.sync.dma_start(afT[:], afT_d[:])
                nc.vector.tensor_copy(afTb[:], afT[:])

                # h0 = mean over l of Af  (h2 = 2*h convention)
                nc.vector.tensor_reduce(
                    cT[:], afT[:], axis=mybir.AxisListType.X,
                    op=mybir.AluOpType.add,
                )
                nc.vector.tensor_scalar_mul(cT[:], cT[:], 2.0 / L)
                nc.vector.tensor_copy(hTb[:], cT[:])

                pps = [ppp.tile([128, 1024], FP32, tag=f"pps{gc}", name=f"pps{gc}") for gc in range(4)]
                for hc in range(HC):
                    wsl = wslp.tile([128, G], FP32, tag="wsl")
                    nc.sync.dma_start(wsl[:], wattn_d[:, hc, :])
                    for gc in range(4):
                        for hf in range(2):
                            nc.tensor.matmul(
                                pps[gc][:, hf * 512:(hf + 1) * 512],
                                afT[:, hc, :, :],
                                wsl[
                                    :,
                                    gc * 1024 + hf * 512:gc * 1024 + (hf + 1) * 512,
                                ],
                                start=(hc == 0),
                                stop=(hc == HC - 1),
                            )
                for gc in range(4):
                    nc.vector.tensor_copy(
                        p_sb[:, gc * 1024:(gc + 1) * 1024], pps[gc][:]
                    )

            # ---------- xWx^T into SBUF bf16 (f32r, one-time) ----------
            with (
                tc.tile_pool(name="xwx", bufs=1) as xp,
                tc.tile_pool(name="xwxs", bufs=2) as xsp,
                tc.tile_pool(name="xwpsum", bufs=1, space="PSUM") as xpp,
            ):
                xT_r = xp.tile([128, DC, NL, timesteps], F32R)
                for dc in range(DC):
                    st2 = xsp.tile([128, NL * timesteps], FP32, tag="stage2")
                    nc.sync.dma_start(
                        st2[:], xT_d[:, dc, :, :].rearrange("p n t -> p (n t)")
                    )
                    nc.vector.tensor_copy(
                        xT_r[:, dc, :, :].rearrange("p n t -> p (n t)"), st2[:]
                    )
                ncols = NL * timesteps  # 1024
                col_chunks = [(s, min(s + 512, ncols)) for s in range(0, ncols, 512)]
                for mg in range(GM // 4):  # groups of 4 gate-chunks
                    xwg = [
                        xpp.tile([128, ncols], FP32, tag=f"xw{i}", name=f"xw{i}")
                        for i in range(4)
                    ]
                    for dc in range(DC):
                        st = xsp.tile([128, 512], FP32, tag="stage")
                        nc.sync.dma_start(
                            st[:], wx_d[:, dc, mg * 512:(mg + 1) * 512]
                        )
                        wxr = xsp.tile([128, 512], F32R, tag="wxr")
                        nc.vector.tensor_copy(wxr[:], st[:])
                        for i in range(4):
                            for (lo, hi) in col_chunks:
                                nc.tensor.matmul(
                                    xwg[i][:, lo:hi],
                                    wxr[:, i * 128:(i + 1) * 128],
                                    xT_r[:, dc, :, :].rearrange(
                                        "p n t -> p (n t)"
                                    )[:, lo:hi],
                                    start=(dc == 0),
                                    stop=(dc == DC - 1),
                                )
                    for i in range(4):
                        m = mg * 4 + i
                        nc.vector.tensor_scalar_add(
                            xwxt[:, m, :, :].rearrange("p n t -> p (n t)"),
                            xwg[i][:],
                            bias_sb[:, m:m + 1],
                        )

            # Wh load (bf16, direct)
            nc.sync.dma_start(wh_sb[:], wh_d[:])

            # ---------- recurrence ----------
            with (
                tc.tile_pool(name="step", bufs=2) as sp,
                tc.tile_pool(name="gpsum", bufs=1, space="PSUM") as gp,
                tc.tile_pool(name="spsum", bufs=1, space="PSUM") as ssp,
            ):
                with tc.For_i(0, timesteps, 1, staggered_reset=True) as ti:
                    aT = gp.tile([128, GM, NL], FP32, tag="aT")
                    uTq = [
                        gp.tile([128, HC, NL], FP32, tag=f"uT{i}", name=f"uT{i}")
                        for i in range(4)
                    ]

                    def wh_block(ms):
                        for m in ms:
                            for kc in range(HC):
                                nc.tensor.matmul(
                                    aT[:, m, :],
                                    wh_sb[:, kc, m * 128:(m + 1) * 128],
                                    hTb[:, kc, :],
                                    start=(kc == 0),
                                    stop=(kc == HC - 1),
                                )

                    def e_block(qi):
                        for j in range(HC):
                            m = qi * HC + j
                            nc.tensor.matmul(
                                uTq[qi][:, j, :],
                                p_sb[:, m * 128:(m + 1) * 128],
                                ee[:],
                                start=True,
                                stop=True,
                            )

                    # ----- scores, partition-major: scp[(n,l), n'] -----
                    scp = ssp.tile([128, NL], FP32, tag="scp")
                    for kc in range(HC):
                        nc.tensor.matmul(
                            scp[:],
                            afTb[:, kc, :, :].rearrange("p n l -> p (n l)"),
                            hTb[:, kc, :],
                            start=(kc == 0),
                            stop=(kc == HC - 1),
                        )
                    # diagonal-block extract: s_col[p] = sum_n mask[p,n]*scp[p,n]
                    junk = sp.tile([128, NL], FP32, tag="junk")
                    s_col = sp.tile([128, 1], FP32, tag="s_col")
                    nc.vector.scalar_tensor_tensor(
                        junk[:], scp[:], 1.0, mask_sb[:],
                        mybir.AluOpType.mult, mybir.AluOpType.mult,
                        accum_out=s_col[:],
                    )
                    # exp via tanh: e = (1+t)/(1-t), t = tanh(s/(2*64*sqrtH^-1...))
                    th = sp.tile([128, 1], FP32, tag="th")
                    nc.scalar.activation(
                        th[:], s_col[:], mybir.ActivationFunctionType.Tanh,
                        scale=INV_SQRT_H / 4.0,
                    )
                    den = sp.tile([128, 1], FP32, tag="den")
                    nc.vector.scalar_tensor_tensor(
                        den[:], th[:], -1.0, ones_sb[:],
                        mybir.AluOpType.mult, mybir.AluOpType.add,
                    )
                    rec = sp.tile([128, 1], FP32, tag="rec")
                    nc.vector.reciprocal(rec[:], den[:])
                    e_col = sp.tile([128, 1], FP32, tag="e_col")
                    nc.vector.scalar_tensor_tensor(
                        e_col[:], th[:], 1.0, rec[:],
                        mybir.AluOpType.add, mybir.AluOpType.mult,
                    )

                    wh_block(range(0, 16))

                    # per-sample sums replicated partition-major
                    zsp = ssp.tile([128, 1], FP32, tag="zsp")
                    nc.tensor.matmul(
                        zsp[:], bmask_sb[:], e_col[:], start=True, stop=True
                    )
                    rz = sp.tile([128, 1], FP32, tag="rz")
                    nc.vector.reciprocal(rz[:], zsp[:])
                    en = sp.tile([128, 1], FP32, tag="en")
                    nc.vector.tensor_tensor(
                        en[:], e_col[:], rz[:], mybir.AluOpType.mult
                    )
                    ee = sp.tile([128, NL], BF16, tag="ee")
                    nc.vector.tensor_scalar_mul(ee[:], mask_sb[:], en[:])

                    wh_block(range(16, 32))

                    spre = sp.tile([128, GM, NL], FP32, tag="spre")
                    gs = sp.tile([128, GM, NL], FP32, tag="gs")
                    gl = gs[:].rearrange("p m n -> p (m n)")
                    q = HC * NL  # 64 columns per gate

                    def gate_block(qi, scale):
                        lo, hi = qi * HC, (qi + 1) * HC
                        nc.vector.tensor_tensor(
                            spre[:, lo:hi, :].rearrange("p m n -> p (m n)").unsqueeze(2),
                            aT[:, lo:hi, :].rearrange("p m n -> p (m n)").unsqueeze(2),
                            xwxt[:, lo:hi, :, bass.ds(ti, 1)].rearrange(
                                "p m n t -> p (m n) t"
                            ),
                            mybir.AluOpType.add,
                        )
                        nc.vector.tensor_tensor(
                            spre[:, lo:hi, :], spre[:, lo:hi, :], uTq[qi][:],
                            mybir.AluOpType.add,
                        )
                        nc.scalar.activation(
                            gl[:, qi * q:(qi + 1) * q],
                            spre[:, lo:hi, :].rearrange("p m n -> p (m n)"),
                            mybir.ActivationFunctionType.Tanh, scale=scale,
                        )

                    cflat = cT[:].rearrange("p c n -> p (c n)")
                    # quarter order g, i, f, o: c-chain hides under the o quarter
                    e_block(3)
                    gate_block(3, 2.0)          # g
                    e_block(0)
                    gate_block(0, 1.0)          # i
                    t2 = sp.tile([128, HC * NL], FP32, tag="t2")
                    nc.vector.scalar_tensor_tensor(
                        t2[:], gl[:, 0 * q:1 * q], 1.0, gl[:, 3 * q:4 * q],
                        mybir.AluOpType.add, mybir.AluOpType.mult,
                    )
                    e_block(1)
                    gate_block(1, 1.0)          # f
                    t1 = sp.tile([128, HC * NL], FP32, tag="t1")
                    nc.vector.scalar_tensor_tensor(
                        t1[:], gl[:, 1 * q:2 * q], 1.0, cflat,
                        mybir.AluOpType.add, mybir.AluOpType.mult,
                    )
                    nc.vector.scalar_tensor_tensor(
                        cflat, t1[:], 0.5, t2[:],
                        mybir.AluOpType.mult, mybir.AluOpType.add,
                    )
                    tc_t = sp.tile([128, HC * NL], FP32, tag="tct")
                    nc.scalar.activation(
                        tc_t[:], cflat, mybir.ActivationFunctionType.Tanh, scale=0.5
                    )
                    e_block(2)
                    gate_block(2, 1.0)          # o
                    # h2 = (to+1)*tanh(c)  (bf16 directly)
                    nc.vector.scalar_tensor_tensor(
                        hTb[:].rearrange("p c n -> p (c n)"),
                        gl[:, 2 * q:3 * q], 1.0, tc_t[:],
                        mybir.AluOpType.add, mybir.AluOpType.mult,
                    )
                    nc.sync.dma_start(
                        out_d[bass.ds(ti, 1), :, :, :].rearrange(
                            "t p c n -> p (t c) n"
                        ),
                        hTb[:],
                    )

    nc.finalize()
    return nc


def prep_inputs(x, A, Wx, Wh, Wattn, b):
    """Host-side reshapes to device layouts; returns per-core input maps."""
    x = np.asarray(x, dtype=np.float32)
    A = np.asarray(A, dtype=np.float32)
    Wx = np.asarray(Wx, dtype=np.float32)
    Wh = np.asarray(Wh, dtype=np.float32)
    Wattn = np.asarray(Wattn, dtype=np.float32)
    b = np.asarray(b, dtype=np.float32)
    timesteps = x.shape[1]

    # weight layouts [p, kc, g] with k = kc*128 + p
    wx_h = np.ascontiguousarray(0.5 * Wx.reshape(DC, 128, G).transpose(1, 0, 2))
    wh_h = np.ascontiguousarray(
        (0.25 * Wh.reshape(HC, 128, G).transpose(1, 0, 2)).astype(ml_dtypes.bfloat16)
    )
    wattn_h = np.ascontiguousarray(0.5 * Wattn.reshape(HC, 128, G).transpose(1, 0, 2))
    b_h = np.ascontiguousarray(0.5 * b.reshape(GM, 128).T)  # [p, m]
    mask_h = np.zeros((128, NL), dtype=np.float32)
    for p in range(128):
        mask_h[p, p // L] = 1.0
    bmask_h = (np.arange(128)[:, None] // L == np.arange(128)[None, :] // L).astype(
        np.float32
    )

    in_maps = []
    for c in range(NC):
        xs = x[c * NL:(c + 1) * NL]          # (8, T, 512)
        As = A[c * NL:(c + 1) * NL].reshape(NL, H, L)  # (8, 1024, 16)
        # xT [p, dc, n, t] = x[n, t, dc*128+p]
        xT_h = np.ascontiguousarray(
            xs.reshape(NL, timesteps, DC, 128).transpose(3, 2, 0, 1)
        )
        # afT [p, hc, n, l] = Af[n, hc*128+p, l]
        afT_h = np.ascontiguousarray(
            As.reshape(NL, HC, 128, L).transpose(2, 1, 0, 3)
        )
        in_maps.append(
            {
                "xT": xT_h,
                "afT": afT_h,
                "wx": wx_h,
                "wh": wh_h,
                "wattn": wattn_h,
                "bias": b_h,
                "mask": mask_h,
                "bmask": bmask_h,
            }
        )
    return in_maps


_NC_CACHE = {}


def kernel(x, A, Wx, Wh, Wattn, b, trace=False):
    timesteps = x.shape[1]
    key = timesteps
    if key not in _NC_CACHE:
        _NC_CACHE[key] = build_nc(timesteps)
    nc = _NC_CACHE[key]
    in_maps = prep_inputs(x, A, Wx, Wh, Wattn, b)
    res = run_bass_kernel_spmd(nc, in_maps, list(range(NC)), trace=trace)
    outs = []
    for c in range(NC):
        hsT = res.results[c]["hsT"]  # (T, 128, HC, NL)
        # out[n, t, hc*128+p] = hsT[t, p, hc, n]
        outs.append(0.5 * hsT.astype(np.float32).transpose(3, 0, 2, 1).reshape(NL, timesteps, H))
    full = np.concatenate(outs, axis=0).astype(np.float32)
    kernel.last_result = res
    return full


# revision 24
# speedup vs baseline: 1.1526x; 1.0003x over previous
"""Trainium2 Bass kernel for the attention-LSTM captioning RNN.

Problem (per full batch): x(64,128,512), A(64,1024,4,4), Wx(512,4096),
Wh(1024,4096), Wattn(1024,4096), b(4096) -> h-sequence (64,128,1024).

Strategy: data-parallel over N across 8 cores (8 samples/core, weights
replicated).  Per core:
  - precompute P[(n,l),g] = Af[n,:,l] @ Wattn  (PE, fp32) -> SBUF bf16
  - precompute xWx^T (gate-major) once (PE, f32r) -> SBUF bf16, indexed
    per step with a strided AP (no per-step DMA)
  - recurrence in transposed ("a^T") layout: gates live on 128 partitions
    (partition = gate-col % 128), batch (8) on the free dim.  Wh is the
    stationary operand (bf16, FWL), h^T the 8-wide moving operand.
    Attention is folded in as a second accumulating matmul with a
    block-diagonal softmax-weight matrix E (128x8) against stationary P.
Host numpy does all layout transposes (free: not timed on device).
"""

import math
import sys

sys.path.insert(0, "/root/shim")
sys.path.insert(0, "/opt/trn_rl_repo")

import numpy as np
import ml_dtypes

try:
    import antenv

    if "/root/shim/antenv" not in list(antenv.__path__):
        antenv.__path__.append("/root/shim/antenv")
except Exception:
    pass

import concourse.bass as bass
import concourse.bacc as bacc
import concourse.mybir as mybir
from concourse.tile import TileContext
from concourse.bass_utils import run_bass_kernel_spmd

FP32 = mybir.dt.float32
F32R = mybir.dt.float32r
BF16 = mybir.dt.bfloat16

# Problem constants (hardcoded per harness contract)
N, T, D, H = 64, 128, 512, 1024
NC = 8            # cores
NL = N // NC      # samples per core = 8
G = 4 * H         # 4096 gate columns
L = 16            # attention locations
HC = H // 128     # 8 h-chunks
GM = G // 128     # 32 gate-col chunks
DC = D // 128     # 4 d-chunks
INV_SQRT_H = 1.0 / math.sqrt(H)


def build_nc(timesteps=T):
    nc = bacc.Bacc()

    # ---- DRAM I/O (host-prepped layouts) ----
    xT_d = nc.dram_tensor("xT", [128, DC, NL, timesteps], FP32, kind="ExternalInput")
    afT_d = nc.dram_tensor("afT", [128, HC, NL, L], FP32, kind="ExternalInput")
    wx_d = nc.dram_tensor("wx", [128, DC, G], FP32, kind="ExternalInput")
    wh_d = nc.dram_tensor("wh", [128, HC, G], BF16, kind="ExternalInput")
    wattn_d = nc.dram_tensor("wattn", [128, HC, G], FP32, kind="ExternalInput")
    b_d = nc.dram_tensor("bias", [128, GM], FP32, kind="ExternalInput")
    mask_d = nc.dram_tensor("mask", [128, NL], FP32, kind="ExternalInput")
    bmask_d = nc.dram_tensor("bmask", [128, 128], FP32, kind="ExternalInput")
    out_d = nc.dram_tensor("hsT", [timesteps, 128, HC, NL], BF16, kind="ExternalOutput")

    with TileContext(nc) as tc:
        # ---------- persistent SBUF ----------
        with tc.tile_pool(name="persist", bufs=1) as pp:
            afTb = pp.tile([128, HC, NL, L], BF16)     # Af^T bf16, (hc,n,l) free
            p_sb = pp.tile([128, G], BF16)             # P[(n,l), g]
            wh_sb = pp.tile([128, HC, G], BF16)        # Wh tiles
            xwxt = pp.tile([128, GM, NL, timesteps], BF16)  # xWx^T (+bias)
            bias_sb = pp.tile([128, GM], FP32)
            mask_sb = pp.tile([128, NL], FP32)
            ones_sb = pp.tile([128, 1], FP32)          # per-partition ones col
            bmask_sb = pp.tile([128, 128], FP32)       # 16-block partition mask
            hTb = pp.tile([128, HC, NL], BF16)         # h^T bf16 (2h)
            cT = pp.tile([128, HC, NL], FP32)

            nc.sync.dma_start(bias_sb[:], b_d[:])
            nc.sync.dma_start(mask_sb[:], mask_d[:])
            nc.vector.memset(ones_sb[:], 1.0)
            nc.sync.dma_start(bmask_sb[:], bmask_d[:])

            # ---------- P = Af^T @ Wattn  (fp32, one-time) ----------
            with (
                tc.tile_pool(name="wattn", bufs=1) as wap,
                tc.tile_pool(name="wsl", bufs=2) as wslp,
                tc.tile_pool(name="ppsum", bufs=1, space="PSUM") as ppp,
            ):
                afT = wap.tile([128, HC, NL, L], FP32)
                nc.sync.dma_start(afT[:], afT_d[:])
                nc.vector.tensor_copy(afTb[:], afT[:])

                # h0 = mean over l of Af  (h2 = 2*h convention)
                nc.vector.tensor_reduce(
                    cT[:], afT[:], axis=mybir.AxisListType.X,
                    op=mybir.AluOpType.add,
                )
                nc.vector.tensor_scalar_mul(cT[:], cT[:], 2.0 / L)
                nc.vector.tensor_copy(hTb[:], cT[:])

                pps = [ppp.tile([128, 1024], FP32, tag=f"pps{gc}", name=f"pps{gc}") for gc in range(4)]
                for hc in range(HC):
                    wsl = wslp.tile([128, G], FP32, tag="wsl")
                    nc.sync.dma_start(wsl[:], wattn_d[:, hc, :])
                    for gc in range(4):
                        for hf in range(2):
                            nc.tensor.matmul(
                                pps[gc][:, hf * 512:(hf + 1) * 512],
                                afT[:, hc, :, :],
                                wsl[
                                    :,
                                    gc * 1024 + hf * 512:gc * 1024 + (hf + 1) * 512,
                                ],
                                start=(hc == 0),
                                stop=(hc == HC - 1),
                            )
                for gc in range(4):
                    nc.vector.tensor_copy(
                        p_sb[:, gc * 1024:(gc + 1) * 1024], pps[gc][:]
                    )

            # ---------- xWx^T into SBUF bf16 (f32r, one-time) ----------
            with (
                tc.tile_pool(name="xwx", bufs=1) as xp,
                tc.tile_pool(name="xwxs", bufs=2) as xsp,
                tc.tile_pool(name="xwpsum", bufs=1, space="PSUM") as xpp,
            ):
                xT_r = xp.tile([128, DC, NL, timesteps], F32R)
                for dc in range(DC):
                    st2 = xsp.tile([128, NL * timesteps], FP32, tag="stage2")
                    nc.sync.dma_start(
                        st2[:], xT_d[:, dc, :, :].rearrange("p n t -> p (n t)")
                    )
                    nc.vector.tensor_copy(
                        xT_r[:, dc, :, :].rearrange("p n t -> p (n t)"), st2[:]
                    )
                ncols = NL * timesteps  # 1024
                col_chunks = [(s, min(s + 512, ncols)) for s in range(0, ncols, 512)]
                for mg in range(GM // 4):  # groups of 4 gate-chunks
                    xwg = [
                        xpp.tile([128, ncols], FP32, tag=f"xw{i}", name=f"xw{i}")
                        for i in range(4)
                    ]
                    for dc in range(DC):
                        st = xsp.tile([128, 512], FP32, tag="stage")
                        nc.sync.dma_start(
                            st[:], wx_d[:, dc, mg * 512:(mg + 1) * 512]
                        )
                        wxr = xsp.tile([128, 512], F32R, tag="wxr")
                        nc.vector.tensor_copy(wxr[:], st[:])
                        for i in range(4):
                            for (lo, hi) in col_chunks:
                                nc.tensor.matmul(
                                    xwg[i][:, lo:hi],
                                    wxr[:, i * 128:(i + 1) * 128],
                                    xT_r[:, dc, :, :].rearrange(
                                        "p n t -> p (n t)"
                                    )[:, lo:hi],
                                    start=(dc == 0),
                                    stop=(dc == DC - 1),
                                )
                    for i in range(4):
                        m = mg * 4 + i
                        nc.vector.tensor_scalar_add(
                            xwxt[:, m, :, :].rearrange("p n t -> p (n t)"),
                            xwg[i][:],
                            bias_sb[:, m:m + 1],
                        )

            # Wh load (bf16, direct)
            nc.sync.dma_start(wh_sb[:], wh_d[:])

            # ---------- recurrence ----------
            with (
                tc.tile_pool(name="step", bufs=2) as sp,
                tc.tile_pool(name="gpsum", bufs=1, space="PSUM") as gp,
                tc.tile_pool(name="spsum", bufs=1, space="PSUM") as ssp,
            ):
                with tc.For_i(0, timesteps, 1, staggered_reset=True) as ti:
                    aT = gp.tile([128, GM, NL], FP32, tag="aT")
                    uTq = [
                        gp.tile([128, HC, NL], FP32, tag=f"uT{i}", name=f"uT{i}")
                        for i in range(4)
                    ]

                    def wh_block(ms):
                        for m in ms:
                            for kc in range(HC):
                                nc.tensor.matmul(
                                    aT[:, m, :],
                                    wh_sb[:, kc, m * 128:(m + 1) * 128],
                                    hTb[:, kc, :],
                                    start=(kc == 0),
                                    stop=(kc == HC - 1),
                                )

                    def e_block(qi):
                        for j in range(HC):
                            m = qi * HC + j
                            nc.tensor.matmul(
                                uTq[qi][:, j, :],
                                p_sb[:, m * 128:(m + 1) * 128],
                                ee[:],
                                start=True,
                                stop=True,
                            )

                    # ----- scores, partition-major: scp[(n,l), n'] -----
                    scp = ssp.tile([128, NL], FP32, tag="scp")
                    for kc in range(HC):
                        nc.tensor.matmul(
                            scp[:],
                            afTb[:, kc, :, :].rearrange("p n l -> p (n l)"),
                            hTb[:, kc, :],
                            start=(kc == 0),
                            stop=(kc == HC - 1),
                        )
                    # diagonal-block extract: s_col[p] = sum_n mask[p,n]*scp[p,n]
                    junk = sp.tile([128, NL], FP32, tag="junk")
                    s_col = sp.tile([128, 1], FP32, tag="s_col")
                    nc.vector.scalar_tensor_tensor(
                        junk[:], scp[:], 1.0, mask_sb[:],
                        mybir.AluOpType.mult, mybir.AluOpType.mult,
                        accum_out=s_col[:],
                    )
                    # exp via tanh: e = (1+t)/(1-t), t = tanh(s/(2*64*sqrtH^-1...))
                    th = sp.tile([128, 1], FP32, tag="th")
                    nc.scalar.activation(
                        th[:], s_col[:], mybir.ActivationFunctionType.Tanh,
                        scale=INV_SQRT_H / 4.0,
                    )
                    den = sp.tile([128, 1], FP32, tag="den")
                    nc.vector.scalar_tensor_tensor(
                        den[:], th[:], -1.0, ones_sb[:],
                        mybir.AluOpType.mult, mybir.AluOpType.add,
                    )
                    rec = sp.tile([128, 1], FP32, tag="rec")
                    nc.vector.reciprocal(rec[:], den[:])
                    e_col = sp.tile([128, 1], FP32, tag="e_col")
                    nc.vector.scalar_tensor_tensor(
                        e_col[:], th[:], 1.0, rec[:],
                        mybir.AluOpType.add, mybir.AluOpType.mult,
                    )

                    wh_block(range(0, 24))

                    # per-sample sums replicated partition-major
                    zsp = ssp.tile([128, 1], FP32, tag="zsp")
                    nc.tensor.matmul(
                        zsp[:], bmask_sb[:], e_col[:], start=True, stop=True
                    )
                    rz = sp.tile([128, 1], FP32, tag="rz")
                    nc.vector.reciprocal(rz[:], zsp[:])
                    en = sp.tile([128, 1], FP32, tag="en")
                    nc.vector.tensor_tensor(
                        en[:], e_col[:], rz[:], mybir.AluOpType.mult
                    )
                    ee = sp.tile([128, NL], BF16, tag="ee")
                    nc.vector.tensor_scalar_mul(ee[:], mask_sb[:], en[:])

                    wh_block(range(24, 32))

                    spre = sp.tile([128, GM, NL], FP32, tag="spre")
                    gs = sp.tile([128, GM, NL], FP32, tag="gs")
                    gl = gs[:].rearrange("p m n -> p (m n)")
                    q = HC * NL  # 64 columns per gate

                    def gate_block(qi, scale):
                        lo, hi = qi * HC, (qi + 1) * HC
                        nc.vector.tensor_tensor(
                            spre[:, lo:hi, :].rearrange("p m n -> p (m n)").unsqueeze(2),
                            aT[:, lo:hi, :].rearrange("p m n -> p (m n)").unsqueeze(2),
                            xwxt[:, lo:hi, :, bass.ds(ti, 1)].rearrange(
                                "p m n t -> p (m n) t"
                            ),
                            mybir.AluOpType.add,
                        )
                        nc.vector.tensor_tensor(
                            spre[:, lo:hi, :], spre[:, lo:hi, :], uTq[qi][:],
                            mybir.AluOpType.add,
                        )
                        nc.scalar.activation(
                            gl[:, qi * q:(qi + 1) * q],
                            spre[:, lo:hi, :].rearrange("p m n -> p (m n)"),
                            mybir.ActivationFunctionType.Tanh, scale=scale,
                        )

                    cflat = cT[:].rearrange("p c n -> p (c n)")
                    # quarter order g, i, f, o: c-chain hides under the o quarter
                    e_block(3)
                    gate_block(3, 1.0)          # g
                    e_block(0)
                    gate_block(0, 1.0)          # i
                    t2 = sp.tile([128, HC * NL], FP32, tag="t2")
                    nc.vector.scalar_tensor_tensor(
                        t2[:], gl[:, 0 * q:1 * q], 1.0, gl[:, 3 * q:4 * q],
                        mybir.AluOpType.add, mybir.AluOpType.mult,
                    )
                    e_block(1)
                    gate_block(1, 1.0)          # f
                    t1 = sp.tile([128, HC * NL], FP32, tag="t1")
                    nc.vector.scalar_tensor_tensor(
                        t1[:], gl[:, 1 * q:2 * q], 1.0, cflat,
                        mybir.AluOpType.add, mybir.AluOpType.mult,
                    )
                    nc.vector.scalar_tensor_tensor(
                        cflat, t1[:], 0.5, t2[:],
                        mybir.AluOpType.mult, mybir.AluOpType.add,
                    )
                    tc_t = sp.tile([128, HC * NL], FP32, tag="tct")
                    nc.scalar.activation(
                        tc_t[:], cflat, mybir.ActivationFunctionType.Tanh, scale=0.5
                    )
                    e_block(2)
                    gate_block(2, 1.0)          # o
                    # h2 = (to+1)*tanh(c)  (bf16 directly)
                    nc.vector.scalar_tensor_tensor(
                        hTb[:].rearrange("p c n -> p (c n)"),
                        gl[:, 2 * q:3 * q], 1.0, tc_t[:],
                        mybir.AluOpType.add, mybir.AluOpType.mult,
                    )
                    nc.sync.dma_start(
                        out_d[bass.ds(ti, 1), :, :, :].rearrange(
                            "t p c n -> p (t c) n"
                        ),
                        hTb[:],
                    )

    nc.finalize()
    return nc


def prep_inputs(x, A, Wx, Wh, Wattn, b):
    """Host-side reshapes to device layouts; returns per-core input maps."""
    x = np.asarray(x, dtype=np.float32)
    A = np.asarray(A, dtype=np.float32)
    Wx = np.asarray(Wx, dtype=np.float32)
    Wh = np.asarray(Wh, dtype=np.float32)
    Wattn = np.asarray(Wattn, dtype=np.float32)
    b = np.asarray(b, dtype=np.float32)
    timesteps = x.shape[1]

    # weight layouts [p, kc, g] with k = kc*128 + p
    # per-gate-column scaling: i/f/o columns carry a 0.5 (tanh half-angle
    # trick), g columns stay full-scale; Wh gets an extra 0.5 (h2 = 2h).
    gsc = np.ones((G,), np.float32) * 0.5
    gsc[3 * H:] = 1.0
    wx_h = np.ascontiguousarray((gsc * Wx).reshape(DC, 128, G).transpose(1, 0, 2))
    wh_h = np.ascontiguousarray(
        ((0.5 * gsc) * Wh).reshape(HC, 128, G).transpose(1, 0, 2).astype(
            ml_dtypes.bfloat16
        )
    )
    wattn_h = np.ascontiguousarray(
        (gsc * Wattn).reshape(HC, 128, G).transpose(1, 0, 2)
    )
    b_h = np.ascontiguousarray((gsc * b).reshape(GM, 128).T)  # [p, m]
    mask_h = np.zeros((128, NL), dtype=np.float32)
    for p in range(128):
        mask_h[p, p // L] = 1.0
    bmask_h = (np.arange(128)[:, None] // L == np.arange(128)[None, :] // L).astype(
        np.float32
    )

    in_maps = []
    for c in range(NC):
        xs = x[c * NL:(c + 1) * NL]          # (8, T, 512)
        As = A[c * NL:(c + 1) * NL].reshape(NL, H, L)  # (8, 1024, 16)
        # xT [p, dc, n, t] = x[n, t, dc*128+p]
        xT_h = np.ascontiguousarray(
            xs.reshape(NL, timesteps, DC, 128).transpose(3, 2, 0, 1)
        )
        # afT [p, hc, n, l] = Af[n, hc*128+p, l]
        afT_h = np.ascontiguousarray(
            As.reshape(NL, HC, 128, L).transpose(2, 1, 0, 3)
        )
        in_maps.append(
            {
                "xT": xT_h,
                "afT": afT_h,
                "wx": wx_h,
                "wh": wh_h,
                "wattn": wattn_h,
                "bias": b_h,
                "mask": mask_h,
                "bmask": bmask_h,
            }
        )
    return in_maps


_NC_CACHE = {}


def kernel(x, A, Wx, Wh, Wattn, b, trace=False):
    timesteps = x.shape[1]
    key = timesteps
    if key not in _NC_CACHE:
        _NC_CACHE[key] = build_nc(timesteps)
    nc = _NC_CACHE[key]
    in_maps = prep_inputs(x, A, Wx, Wh, Wattn, b)
    res = run_bass_kernel_spmd(nc, in_maps, list(range(NC)), trace=trace)
    outs = []
    for c in range(NC):
        hsT = res.results[c]["hsT"]  # (T, 128, HC, NL)
        # out[n, t, hc*128+p] = hsT[t, p, hc, n]
        outs.append(0.5 * hsT.astype(np.float32).transpose(3, 0, 2, 1).reshape(NL, timesteps, H))
    full = np.concatenate(outs, axis=0).astype(np.float32)
    kernel.last_result = res
    return full
